# revision 11
# baseline (speedup 1.0000x reference)
"""Distributed Trainium2 Bass kernel for GQA attention prefill.

Problem: B=2, S=2048, D=4096, 32 q heads, 8 kv heads, head_dim=128, RoPE,
causal mask, start_pos=0.

Sharding (8 cores): DP2 over batch x TP4 over heads.  Core c = b*4 + g gets
batch b, q-heads 8g..8g+7, kv-heads 2g..2g+1, wo rows for those q-heads.
Each core computes a partial [S, D] output (bf16); the host sums the 4
partials per batch (the row-parallel wo unshard).

All layout work happens on the host: x arrives pre-transposed and pre-cast
to bf16 (so the kernel never transposes or casts x on device), weights
arrive bf16 pre-tiled, RoPE tables arrive precomputed in their on-chip
[128, S] layout.

On-core dataflow per half (1024 query positions):
  load xT bf16 tiles [128, 1024] (one per k-chunk)
  QKV projection (bf16 matmuls, head-dim columns pre-permuted [even|odd])
  RoPE applied on the projection PSUM (cross-partition DVE ops)
  scoresT[t,s] = K^T.T @ Q^T, causal mask via on-chip affine_select tile,
  exp on ACT (no max subtraction; scores are ~N(0,1)),
  outT += V^T.T @ P^T accumulated over T-chunks, rowsum via ones-matmul,
  normalize, project with wo (bf16, streamed), DMA bf16 partial out.
"""

import math

import numpy as np
import ml_dtypes

import concourse.bass as bass  # noqa: F401  (bass types via bacc)
import concourse.mybir as mybir
from concourse import bacc
from concourse.bass_utils import run_bass_kernel_spmd
from concourse.tile import TileContext

F32 = mybir.dt.float32
BF16 = mybir.dt.bfloat16
NPBF = ml_dtypes.bfloat16

B, S, D = 2, 2048, 4096
NH, NKV, HD = 32, 8, 128
NCORES = 8
TPG = 4                  # tensor-parallel groups
NQL = NH // TPG          # 8 local q heads
NKVL = NKV // TPG        # 2 local kv heads
SCW = 512                # s-chunk width
HW = S // 2              # half width (1024)
NKC = D // 128           # 32 contraction chunks for projections
NTC = S // 128           # 16 T-chunks (key positions)
SCALE = 1.0 / math.sqrt(HD)
NEG = -1e9


def _build():
    nc = bacc.Bacc("TRN2", target_bir_lowering=False, debug=False,
                   num_devices=NCORES)
    # x pre-transposed + pre-cast: [D, S] bf16
    xt_d = nc.declare_dram_parameter("xt", [D, S], BF16, isOutput=False)
    # weights arrive pre-tiled bf16: [128, m-major kc-major cols]
    wq = nc.declare_dram_parameter("wq", [128, NQL * NKC * HD], BF16, isOutput=False)
    wk = nc.declare_dram_parameter("wk", [128, NKVL * NKC * HD], BF16, isOutput=False)
    wv = nc.declare_dram_parameter("wv", [128, NKVL * NKC * HD], BF16, isOutput=False)
    wo = nc.declare_dram_parameter("wo", [128, (D // SCW) * NQL * SCW], BF16, isOutput=False)
    # RoPE tables in on-chip layout [128, S]
    cosp = nc.declare_dram_parameter("cosp", [128, S], BF16, isOutput=False)
    sinp = nc.declare_dram_parameter("sinp", [128, S], BF16, isOutput=False)
    out = nc.declare_dram_parameter("out", [S, D], BF16, isOutput=True)

    NM = NQL + 2 * NKVL
    WBLK = NKC * HD          # weight cols per m-chunk

    with TileContext(nc) as tc:
        with (
            tc.tile_pool(name="const", bufs=1) as const,
            tc.tile_pool(name="big", bufs=1) as big,
            tc.tile_pool(name="sb", bufs=3) as sb,
            tc.tile_pool(name="ps", bufs=1, space="PSUM") as ps,
        ):
            # ---- weight-slice loader (first two pre-issued before consts
            # so the first QKV matmul is not stuck behind bulk DMA) ----
            wsls = {}

            def load_wsl(hf, m):
                if (hf, m) in wsls:
                    return wsls[(hf, m)]
                wsl = sb.tile([128, NKC * HD], BF16, name=f"w{hf}_{m}",
                              tag="wsl", bufs=3)
                if m < NQL:
                    base = wq
                    m0 = m
                elif m < NQL + NKVL:
                    base = wk
                    m0 = m - NQL
                else:
                    base = wv
                    m0 = m - NQL - NKVL
                qw = NKC * HD // 4
                for q4 in range(4):
                    c0 = m0 * NKC * HD + q4 * qw
                    nc.gpsimd.dma_start(out=wsl[:, q4 * qw:(q4 + 1) * qw],
                                        in_=base[:, c0:c0 + qw])
                wsls[(hf, m)] = wsl
                return wsl

            load_wsl(0, NQL)      # first K head: needed by the first matmul
            load_wsl(0, NQL + 1)

            # ---- constants ----
            ident = const.tile([128, 128], BF16, name="ident")
            nc.gpsimd.memset(ident[:], 0.0)
            nc.gpsimd.affine_select(
                out=ident[:], in_=ident[:],
                compare_op=mybir.AluOpType.not_equal, fill=1.0,
                base=0, pattern=[[-1, 128]], channel_multiplier=1,
            )
            ones = const.tile([128, 128], BF16, name="ones")
            nc.gpsimd.memset(ones[:], 1.0)
            maskbig = const.tile([128, 896], F32, name="maskbig")
            nc.gpsimd.memset(maskbig[:], 0.0)
            nc.gpsimd.affine_select(
                out=maskbig[:], in_=maskbig[:],
                compare_op=mybir.AluOpType.is_ge, fill=NEG,
                base=-384, pattern=[[1, 896]], channel_multiplier=-1,
            )
            cos2 = const.tile([128, S], BF16, name="cos2")
            sin2n = const.tile([128, S], BF16, name="sin2n")
            for hh in range(2):
                hsl = slice(hh * (S // 2), (hh + 1) * (S // 2))
                nc.scalar.dma_start(out=cos2[:, hsl], in_=cosp[:, hsl])
                nc.scalar.dma_start(out=sin2n[:, hsl], in_=sinp[:, hsl])

            ksb = big.tile([128, NKVL * S], BF16, name="ksb")
            vsb = big.tile([128, NTC * NKVL * HD], BF16, name="vsb")
            # attention outputs for the full sequence (WO runs once at the end)
            attnT = [big.tile([128, S], BF16, name=f"at{h}") for h in range(NQL)]

            # ---- xT loads: per (hf, kc) one tile [128, 1024] ----
            xts = {}

            def load_xt(hf):
                for kc in range(NKC):
                    t = sb.tile([128, HW], BF16, name=f"xt{hf}_{kc}",
                                tag="xt", bufs=33)
                    for ph in range(2):
                        nc.sync.dma_start(
                            out=t[ph * 64:(ph + 1) * 64, :],
                            in_=xt_d[kc * 128 + ph * 64:kc * 128 + (ph + 1) * 64,
                                     hf * HW:(hf + 1) * HW])
                    xts[(hf, kc)] = t

            load_xt(0)

            for hf in range(2):
                # ---- QKV projection (m outer; K,V heads first, then Q) ----
                qtiles = [None] * NQL
                for m in list(range(NQL, NM)) + list(range(NQL)):
                    wsl = load_wsl(hf, m)
                    if m < NQL:
                        qt = sb.tile([128, HW], BF16, name=f"q{hf}_{m}",
                                     tag=f"q{m}", bufs=1)
                        qtiles[m] = qt
                    for scq in range(2):
                        sc = hf * 2 + scq
                        ssl = slice(sc * SCW, (sc + 1) * SCW)
                        qsl = slice(scq * SCW, (scq + 1) * SCW)
                        pp = ps.tile([128, SCW], F32, name=f"pp{hf}_{m}_{scq}",
                                     tag="proj", bufs=2)
                        for kc in range(NKC):
                            nc.tensor.matmul(
                                pp[:], wsl[:, kc * 128:(kc + 1) * 128],
                                xts[(hf, kc)][:, qsl],
                                start=(kc == 0), stop=(kc == NKC - 1),
                            )
                        if m < NQL + NKVL:
                            if m < NQL:
                                dst = qtiles[m][:, qsl]
                            else:
                                kv = m - NQL
                                dst = ksb[:, kv * S + sc * SCW:kv * S + (sc + 1) * SCW]
                            t1 = sb.tile([128, SCW], BF16, name=f"t1_{hf}_{m}_{scq}",
                                         tag="t1", bufs=2)
                            t2 = sb.tile([128, SCW], BF16, name=f"t2_{hf}_{m}_{scq}",
                                         tag="t2", bufs=2)
                            nc.vector.tensor_tensor(
                                out=t1[0:64, :], in0=pp[64:128, :],
                                in1=sin2n[0:64, ssl], op=mybir.AluOpType.mult)
                            nc.vector.tensor_tensor(
                                out=t1[64:128, :], in0=pp[0:64, :],
                                in1=sin2n[64:128, ssl], op=mybir.AluOpType.mult)
                            nc.vector.tensor_tensor(
                                out=t2[:], in0=pp[:], in1=cos2[:, ssl],
                                op=mybir.AluOpType.mult)
                            nc.vector.tensor_tensor(
                                out=dst, in0=t1[:], in1=t2[:],
                                op=mybir.AluOpType.add)
                        else:
                            kv = m - NQL - NKVL
                            vts = sb.tile([128, SCW], BF16, name=f"vts{hf}_{kv}_{scq}",
                                          tag="vts", bufs=2)
                            nc.vector.tensor_copy(out=vts[:], in_=pp[:])
                            for j in range(SCW // 128):
                                pv = ps.tile([128, 128], BF16,
                                             name=f"pv{hf}_{kv}_{scq}_{j}",
                                             tag="sc", bufs=3)
                                nc.tensor.transpose(
                                    pv[:], vts[:, j * 128:(j + 1) * 128], ident[:])
                                slot = (sc * 4 + j) * NKVL + kv
                                nc.scalar.copy(
                                    out=vsb[:, slot * HD:(slot + 1) * HD], in_=pv[:])

                if hf == 0:
                    load_xt(1)

                # ---- attention for both s-chunks of this half --------------
                for scq in range(2):
                    sc = hf * 2 + scq
                    ntc = 4 * sc + 4
                    for h in range(NQL):
                        kv = h // (NQL // NKVL)
                        po = ps.tile([128, SCW], F32, name=f"po{sc}_{h}", tag="o", bufs=2)
                        pr = ps.tile([128, SCW], F32, name=f"pr{sc}_{h}", tag="r")
                        pts = {}

                        def geom(tcx):
                            # narrow the work to the unmasked s-range:
                            # for partial tiles (tcx >= 4*sc, j = tcx-4*sc)
                            # only s >= j*128 within the chunk survives.
                            j = tcx - 4 * sc
                            off = j * 128 if j > 0 else 0
                            return j, off, SCW - off

                        def scores(tcx):
                            j, off, w = geom(tcx)
                            qs0 = scq * SCW + off
                            pss = ps.tile([128, SCW], F32,
                                          name=f"ps{sc}_{h}_{tcx}", tag="sc", bufs=3)
                            nc.tensor.matmul(
                                pss[:, :w],
                                ksb[:, kv * S + tcx * 128:kv * S + (tcx + 1) * 128],
                                qtiles[h][:, qs0:qs0 + w],
                                start=True, stop=True,
                            )
                            if j >= 0:
                                nc.vector.tensor_tensor(
                                    out=pss[:, :128], in0=pss[:, :128],
                                    in1=maskbig[:, 384:512],
                                    op=mybir.AluOpType.add)
                            pt = sb.tile([128, SCW], BF16, name=f"pt{sc}_{h}_{tcx}",
                                         tag="pt", bufs=4)
                            nc.scalar.activation(
                                pt[:, :w], pss[:, :w],
                                mybir.ActivationFunctionType.Exp, scale=SCALE)
                            pts[tcx] = pt

                        scores(0)
                        if ntc > 1:
                            scores(1)
                        for tcx in range(ntc):
                            if tcx + 2 < ntc:
                                scores(tcx + 2)
                            j, off, w = geom(tcx)
                            pt = pts.pop(tcx)
                            slot = tcx * NKVL + kv
                            nc.tensor.matmul(
                                po[:, off:], vsb[:, slot * HD:(slot + 1) * HD],
                                pt[:, :w],
                                start=(tcx == 0), stop=(tcx == ntc - 1))
                            nc.tensor.matmul(
                                pr[:, off:], ones[:], pt[:, :w],
                                start=(tcx == 0), stop=(tcx == ntc - 1))
                        rec = sb.tile([128, SCW], F32, name=f"rec{sc}_{h}",
                                      tag="rec", bufs=1)
                        rin = sb.tile([128, SCW], F32, name=f"rin{sc}_{h}",
                                      tag="rin", bufs=1)
                        nc.vector.tensor_copy(out=rin[:], in_=pr[:])
                        nc.vector.reciprocal_approx_fast(out=rec[:], in_=rin[:])
                        nc.vector.tensor_tensor(
                            out=attnT[h][:, sc * SCW:(sc + 1) * SCW],
                            in0=po[:], in1=rec[:],
                            op=mybir.AluOpType.mult)

            # ---- output projection, once over the full sequence ------------
            oblk = NQL * SCW
            wots = {}

            def load_wot(dc):
                wot = sb.tile([128, NQL * SCW], BF16, name=f"wot{dc}",
                              tag="wot", bufs=3)
                qw = oblk // 4
                for q4 in range(4):
                    nc.scalar.dma_start(
                        out=wot[:, q4 * qw:(q4 + 1) * qw],
                        in_=wo[:, dc * oblk + q4 * qw:dc * oblk + (q4 + 1) * qw])
                wots[dc] = wot

            load_wot(0)
            load_wot(1)
            for dc in range(D // SCW):
                wot = wots[dc]
                if dc + 2 < D // SCW:
                    load_wot(dc + 2)
                for ssub in range(S // 128):
                    s0 = ssub * 128
                    pd = ps.tile([128, SCW], F32, name=f"pd{dc}_{ssub}",
                                 tag="proj", bufs=2)
                    for kc8 in range(NQL):
                        nc.tensor.matmul(
                            pd[:],
                            attnT[kc8][:, s0:s0 + 128],
                            wot[:, kc8 * SCW:(kc8 + 1) * SCW],
                            start=(kc8 == 0), stop=(kc8 == NQL - 1))
                    os_ = sb.tile([128, SCW], BF16, name=f"os{dc}_{ssub}",
                                  tag="os", bufs=2)
                    nc.vector.tensor_copy(out=os_[:], in_=pd[:])
                    nc.sync.dma_start(
                        out=out[s0:s0 + 128, dc * SCW:(dc + 1) * SCW],
                        in_=os_[:])
    nc.finalize()
    return nc


_NC_CACHE = None


def _get_graph():
    global _NC_CACHE
    if _NC_CACHE is None:
        _NC_CACHE = _build()
    return _NC_CACHE


_PERM = np.concatenate([np.arange(0, HD, 2), np.arange(1, HD, 2)])


def _tile_w(w):
    """[D, M*HD] -> [128, m-major kc-major 128cols] contiguous tiling (bf16)."""
    d, mc = w.shape
    nm = mc // HD
    t = w.reshape(NKC, 128, nm, HD).transpose(1, 2, 0, 3)
    return np.ascontiguousarray(t.reshape(128, nm * NKC * HD).astype(NPBF))


def _tile_wo(w):
    """[NQL*HD, D] -> [128, dc-major kc-major 512cols] (bf16)."""
    t = w.reshape(NQL, 128, D // SCW, SCW).transpose(1, 2, 0, 3)
    return np.ascontiguousarray(
        t.reshape(128, (D // SCW) * NQL * SCW).astype(NPBF))


def _shard_inputs(x, freqs_cos, freqs_sin, wq, wk, wv, wo):
    """Build the 8 per-core input maps (pure numpy prep, nothing on-device)."""
    x = np.asarray(x, dtype=np.float32)
    wq = np.asarray(wq, dtype=np.float32)
    wk = np.asarray(wk, dtype=np.float32)
    wv = np.asarray(wv, dtype=np.float32)
    wo = np.asarray(wo, dtype=np.float32)
    cos = np.asarray(freqs_cos, dtype=np.float32)
    sin = np.asarray(freqs_sin, dtype=np.float32)

    # RoPE tables in on-chip [128, S] layout
    cos2 = np.empty((128, S), np.float32)
    sin2n = np.empty((128, S), np.float32)
    cos2[0:64] = cos.T
    cos2[64:128] = cos.T
    sin2n[0:64] = -sin.T
    sin2n[64:128] = sin.T
    cos2 = cos2.astype(NPBF)
    sin2n = sin2n.astype(NPBF)

    wq4 = wq.reshape(D, NH, HD)
    wk4 = wk.reshape(D, NKV, HD)
    wv4 = wv.reshape(D, NKV, HD)
    wo4 = wo.reshape(NH, HD, D)

    # x transposed + bf16 per batch
    xts = [np.ascontiguousarray(x[b].T.astype(NPBF)) for b in range(B)]

    in_maps = []
    for c in range(NCORES):
        b, g = divmod(c, TPG)
        qh = slice(g * NQL, (g + 1) * NQL)
        kvh = slice(g * NKVL, (g + 1) * NKVL)
        m = {
            "xt": xts[b],
            "wq": _tile_w(wq4[:, qh, :][:, :, _PERM].reshape(D, NQL * HD)),
            "wk": _tile_w(wk4[:, kvh, :][:, :, _PERM].reshape(D, NKVL * HD)),
            "wv": _tile_w(wv4[:, kvh, :].reshape(D, NKVL * HD)),
            "wo": _tile_wo(wo4[qh].reshape(NQL * HD, D)),
            "cosp": cos2,
            "sinp": sin2n,
        }
        in_maps.append(m)
    return in_maps


def kernel(x, start_pos, freqs_cos, freqs_sin, mask, wq, wk, wv, wo,
           cache_k, cache_v):
    x = np.asarray(x)
    in_maps = _shard_inputs(x, freqs_cos, freqs_sin, wq, wk, wv, wo)
    nc = _get_graph()
    res = run_bass_kernel_spmd(nc, in_maps, core_ids=list(range(NCORES)))
    out = np.zeros((B, S, D), dtype=np.float32)
    for b in range(B):
        acc = np.asarray(res.results[b * TPG]["out"]).astype(np.float32)
        for g in range(1, TPG):
            acc += np.asarray(res.results[b * TPG + g]["out"]).astype(np.float32)
        out[b] = acc
    return out


# revision 12
# speedup vs baseline: 1.0015x; 1.0015x over previous
"""Distributed Trainium2 Bass kernel for GQA attention prefill.

Problem: B=2, S=2048, D=4096, 32 q heads, 8 kv heads, head_dim=128, RoPE,
causal mask, start_pos=0.

Sharding (8 cores): DP2 over batch x TP4 over heads.  Core c = b*4 + g gets
batch b, q-heads 8g..8g+7, kv-heads 2g..2g+1, wo rows for those q-heads.
Each core computes a partial [S, D] output (bf16); the host sums the 4
partials per batch (the row-parallel wo unshard).

All layout work happens on the host: x arrives pre-transposed and pre-cast
to bf16 (so the kernel never transposes or casts x on device), weights
arrive bf16 pre-tiled, RoPE tables arrive precomputed in their on-chip
[128, S] layout.

On-core dataflow per half (1024 query positions):
  load xT bf16 tiles [128, 1024] (one per k-chunk)
  QKV projection (bf16 matmuls, head-dim columns pre-permuted [even|odd])
  RoPE applied on the projection PSUM (cross-partition DVE ops)
  scoresT[t,s] = K^T.T @ Q^T, causal mask via on-chip affine_select tile,
  exp on ACT (no max subtraction; scores are ~N(0,1)),
  outT += V^T.T @ P^T accumulated over T-chunks, rowsum via ones-matmul,
  normalize, project with wo (bf16, streamed), DMA bf16 partial out.
"""

import math

import numpy as np
import ml_dtypes

import concourse.bass as bass  # noqa: F401  (bass types via bacc)
import concourse.mybir as mybir
from concourse import bacc
from concourse.bass_utils import run_bass_kernel_spmd
from concourse.tile import TileContext

F32 = mybir.dt.float32
BF16 = mybir.dt.bfloat16
NPBF = ml_dtypes.bfloat16

B, S, D = 2, 2048, 4096
NH, NKV, HD = 32, 8, 128
NCORES = 8
TPG = 4                  # tensor-parallel groups
NQL = NH // TPG          # 8 local q heads
NKVL = NKV // TPG        # 2 local kv heads
SCW = 512                # s-chunk width
HW = S // 2              # half width (1024)
NKC = D // 128           # 32 contraction chunks for projections
NTC = S // 128           # 16 T-chunks (key positions)
SCALE = 1.0 / math.sqrt(HD)
NEG = -1e9


def _build():
    nc = bacc.Bacc("TRN2", target_bir_lowering=False, debug=False,
                   num_devices=NCORES)
    # x pre-transposed + pre-cast: [D, S] bf16
    xt_d = nc.declare_dram_parameter("xt", [D, S], BF16, isOutput=False)
    # weights arrive pre-tiled bf16: [128, m-major kc-major cols]
    wq = nc.declare_dram_parameter("wq", [128, NQL * NKC * HD], BF16, isOutput=False)
    wk = nc.declare_dram_parameter("wk", [128, NKVL * NKC * HD], BF16, isOutput=False)
    wv = nc.declare_dram_parameter("wv", [128, NKVL * NKC * HD], BF16, isOutput=False)
    wo = nc.declare_dram_parameter("wo", [128, (D // SCW) * NQL * SCW], BF16, isOutput=False)
    # RoPE tables in on-chip layout [128, S]
    cosp = nc.declare_dram_parameter("cosp", [128, S], BF16, isOutput=False)
    sinp = nc.declare_dram_parameter("sinp", [128, S], BF16, isOutput=False)
    out = nc.declare_dram_parameter("out", [S, D], BF16, isOutput=True)

    NM = NQL + 2 * NKVL
    WBLK = NKC * HD          # weight cols per m-chunk

    with TileContext(nc) as tc:
        with (
            tc.tile_pool(name="const", bufs=1) as const,
            tc.tile_pool(name="big", bufs=1) as big,
            tc.tile_pool(name="sb", bufs=3) as sb,
            tc.tile_pool(name="ps", bufs=1, space="PSUM") as ps,
        ):
            # ---- weight-slice loader (first two pre-issued before consts
            # so the first QKV matmul is not stuck behind bulk DMA) ----
            wsls = {}

            def load_wsl(hf, m):
                if (hf, m) in wsls:
                    return wsls[(hf, m)]
                wsl = sb.tile([128, NKC * HD], BF16, name=f"w{hf}_{m}",
                              tag="wsl", bufs=3)
                if m < NQL:
                    base = wq
                    m0 = m
                elif m < NQL + NKVL:
                    base = wk
                    m0 = m - NQL
                else:
                    base = wv
                    m0 = m - NQL - NKVL
                qw = NKC * HD // 4
                for q4 in range(4):
                    c0 = m0 * NKC * HD + q4 * qw
                    nc.gpsimd.dma_start(out=wsl[:, q4 * qw:(q4 + 1) * qw],
                                        in_=base[:, c0:c0 + qw])
                wsls[(hf, m)] = wsl
                return wsl

            for m0_ in range(NQL, NQL + 2 * NKVL):
                load_wsl(0, m0_)      # K/V heads: consumed by the grouped pass

            # ---- constants ----
            ident = const.tile([128, 128], BF16, name="ident")
            nc.gpsimd.memset(ident[:], 0.0)
            nc.gpsimd.affine_select(
                out=ident[:], in_=ident[:],
                compare_op=mybir.AluOpType.not_equal, fill=1.0,
                base=0, pattern=[[-1, 128]], channel_multiplier=1,
            )
            ones = const.tile([128, 128], BF16, name="ones")
            nc.gpsimd.memset(ones[:], 1.0)
            maskbig = const.tile([128, 896], F32, name="maskbig")
            nc.gpsimd.memset(maskbig[:], 0.0)
            nc.gpsimd.affine_select(
                out=maskbig[:], in_=maskbig[:],
                compare_op=mybir.AluOpType.is_ge, fill=NEG,
                base=-384, pattern=[[1, 896]], channel_multiplier=-1,
            )
            cos2 = const.tile([128, S], BF16, name="cos2")
            sin2n = const.tile([128, S], BF16, name="sin2n")
            for hh in range(2):
                hsl = slice(hh * (S // 2), (hh + 1) * (S // 2))
                nc.scalar.dma_start(out=cos2[:, hsl], in_=cosp[:, hsl])
                nc.scalar.dma_start(out=sin2n[:, hsl], in_=sinp[:, hsl])

            ksb = big.tile([128, NKVL * S], BF16, name="ksb")
            vsb = big.tile([128, NTC * NKVL * HD], BF16, name="vsb")
            # attention outputs for the full sequence (WO runs once at the end)
            attnT = [big.tile([128, S], BF16, name=f"at{h}") for h in range(NQL)]

            # ---- xT loads: per (hf, kc) one tile [128, 1024] ----
            xts = {}

            def load_xt(hf):
                for kc in range(NKC):
                    t = sb.tile([128, HW], BF16, name=f"xt{hf}_{kc}",
                                tag="xt", bufs=33)
                    for ph in range(2):
                        nc.sync.dma_start(
                            out=t[ph * 64:(ph + 1) * 64, :],
                            in_=xt_d[kc * 128 + ph * 64:kc * 128 + (ph + 1) * 64,
                                     hf * HW:(hf + 1) * HW])
                    xts[(hf, kc)] = t

            load_xt(0)

            def rope_or_v(hf, m, scq, pp, qtiles):
                sc = hf * 2 + scq
                ssl = slice(sc * SCW, (sc + 1) * SCW)
                qsl = slice(scq * SCW, (scq + 1) * SCW)
                if m < NQL + NKVL:
                    if m < NQL:
                        dst = qtiles[m][:, qsl]
                    else:
                        kv = m - NQL
                        dst = ksb[:, kv * S + sc * SCW:kv * S + (sc + 1) * SCW]
                    t1 = sb.tile([128, SCW], BF16, name=f"t1_{hf}_{m}_{scq}",
                                 tag="t1", bufs=2)
                    t2 = sb.tile([128, SCW], BF16, name=f"t2_{hf}_{m}_{scq}",
                                 tag="t2", bufs=2)
                    nc.vector.tensor_tensor(
                        out=t1[0:64, :], in0=pp[64:128, :],
                        in1=sin2n[0:64, ssl], op=mybir.AluOpType.mult)
                    nc.vector.tensor_tensor(
                        out=t1[64:128, :], in0=pp[0:64, :],
                        in1=sin2n[64:128, ssl], op=mybir.AluOpType.mult)
                    nc.vector.tensor_tensor(
                        out=t2[:], in0=pp[:], in1=cos2[:, ssl],
                        op=mybir.AluOpType.mult)
                    nc.vector.tensor_tensor(
                        out=dst, in0=t1[:], in1=t2[:],
                        op=mybir.AluOpType.add)
                else:
                    kv = m - NQL - NKVL
                    vts = sb.tile([128, SCW], BF16, name=f"vts{hf}_{kv}_{scq}",
                                  tag="vts", bufs=2)
                    nc.vector.tensor_copy(out=vts[:], in_=pp[:])
                    for j in range(SCW // 128):
                        pv = ps.tile([128, 128], BF16,
                                     name=f"pv{hf}_{kv}_{scq}_{j}",
                                     tag="sc", bufs=3)
                        nc.tensor.transpose(
                            pv[:], vts[:, j * 128:(j + 1) * 128], ident[:])
                        slot = (sc * 4 + j) * NKVL + kv
                        nc.scalar.copy(
                            out=vsb[:, slot * HD:(slot + 1) * HD], in_=pv[:])

            for hf in range(2):
                # ---- QKV projection (m outer; K,V heads first, then Q) ----
                qtiles = [None] * NQL
                if hf == 0:
                    # grouped pass over the 4 K/V heads: each freshly-DMAed
                    # xt tile feeds 4 accumulators so PE keeps pace with the
                    # initial x fill
                    grp = list(range(NQL, NM))
                    wsl4 = {m: load_wsl(0, m) for m in grp}
                    for scq in range(2):
                        pp4 = {}
                        for gi, m in enumerate(grp):
                            tag = "proj" if gi < 2 else "sc"
                            bufs = 2 if gi < 2 else 3
                            pp4[m] = ps.tile([128, SCW], F32,
                                             name=f"pp0_{m}_{scq}", tag=tag,
                                             bufs=bufs)
                        for kc in range(NKC):
                            for m in grp:
                                nc.tensor.matmul(
                                    pp4[m][:],
                                    wsl4[m][:, kc * 128:(kc + 1) * 128],
                                    xts[(0, kc)][:, scq * SCW:(scq + 1) * SCW],
                                    start=(kc == 0), stop=(kc == NKC - 1),
                                )
                        for m in grp:
                            rope_or_v(0, m, scq, pp4[m], qtiles)
                    morder = list(range(NQL))
                else:
                    morder = list(range(NQL, NM)) + list(range(NQL))
                for m in morder:
                    wsl = load_wsl(hf, m)
                    if m < NQL:
                        qt = sb.tile([128, HW], BF16, name=f"q{hf}_{m}",
                                     tag=f"q{m}", bufs=1)
                        qtiles[m] = qt
                    for scq in range(2):
                        qsl = slice(scq * SCW, (scq + 1) * SCW)
                        pp = ps.tile([128, SCW], F32, name=f"pp{hf}_{m}_{scq}",
                                     tag="proj", bufs=2)
                        for kc in range(NKC):
                            nc.tensor.matmul(
                                pp[:], wsl[:, kc * 128:(kc + 1) * 128],
                                xts[(hf, kc)][:, qsl],
                                start=(kc == 0), stop=(kc == NKC - 1),
                            )
                        rope_or_v(hf, m, scq, pp, qtiles)

                if hf == 0:
                    load_xt(1)

                # ---- attention for both s-chunks of this half --------------
                for scq in range(2):
                    sc = hf * 2 + scq
                    ntc = 4 * sc + 4
                    for h in range(NQL):
                        kv = h // (NQL // NKVL)
                        po = ps.tile([128, SCW], F32, name=f"po{sc}_{h}", tag="o", bufs=2)
                        pr = ps.tile([128, SCW], F32, name=f"pr{sc}_{h}", tag="r")
                        pts = {}

                        def geom(tcx):
                            # narrow the work to the unmasked s-range:
                            # for partial tiles (tcx >= 4*sc, j = tcx-4*sc)
                            # only s >= j*128 within the chunk survives.
                            j = tcx - 4 * sc
                            off = j * 128 if j > 0 else 0
                            return j, off, SCW - off

                        def scores(tcx):
                            j, off, w = geom(tcx)
                            qs0 = scq * SCW + off
                            pss = ps.tile([128, SCW], F32,
                                          name=f"ps{sc}_{h}_{tcx}", tag="sc", bufs=3)
                            nc.tensor.matmul(
                                pss[:, :w],
                                ksb[:, kv * S + tcx * 128:kv * S + (tcx + 1) * 128],
                                qtiles[h][:, qs0:qs0 + w],
                                start=True, stop=True,
                            )
                            if j >= 0:
                                nc.vector.tensor_tensor(
                                    out=pss[:, :128], in0=pss[:, :128],
                                    in1=maskbig[:, 384:512],
                                    op=mybir.AluOpType.add)
                            pt = sb.tile([128, SCW], BF16, name=f"pt{sc}_{h}_{tcx}",
                                         tag="pt", bufs=4)
                            nc.scalar.activation(
                                pt[:, :w], pss[:, :w],
                                mybir.ActivationFunctionType.Exp, scale=SCALE)
                            pts[tcx] = pt

                        scores(0)
                        if ntc > 1:
                            scores(1)
                        for tcx in range(ntc):
                            if tcx + 2 < ntc:
                                scores(tcx + 2)
                            j, off, w = geom(tcx)
                            pt = pts.pop(tcx)
                            slot = tcx * NKVL + kv
                            nc.tensor.matmul(
                                po[:, off:], vsb[:, slot * HD:(slot + 1) * HD],
                                pt[:, :w],
                                start=(tcx == 0), stop=(tcx == ntc - 1))
                            nc.tensor.matmul(
                                pr[:, off:], ones[:], pt[:, :w],
                                start=(tcx == 0), stop=(tcx == ntc - 1))
                        rec = sb.tile([128, SCW], F32, name=f"rec{sc}_{h}",
                                      tag="rec", bufs=1)
                        rin = sb.tile([128, SCW], F32, name=f"rin{sc}_{h}",
                                      tag="rin", bufs=1)
                        nc.vector.tensor_copy(out=rin[:], in_=pr[:])
                        nc.vector.reciprocal_approx_fast(out=rec[:], in_=rin[:])
                        nc.vector.tensor_tensor(
                            out=attnT[h][:, sc * SCW:(sc + 1) * SCW],
                            in0=po[:], in1=rec[:],
                            op=mybir.AluOpType.mult)

            # ---- output projection, once over the full sequence ------------
            oblk = NQL * SCW
            wots = {}

            def load_wot(dc):
                wot = sb.tile([128, NQL * SCW], BF16, name=f"wot{dc}",
                              tag="wot", bufs=2)
                qw = oblk // 4
                for q4 in range(4):
                    nc.scalar.dma_start(
                        out=wot[:, q4 * qw:(q4 + 1) * qw],
                        in_=wo[:, dc * oblk + q4 * qw:dc * oblk + (q4 + 1) * qw])
                wots[dc] = wot

            load_wot(0)
            load_wot(1)
            for dc in range(D // SCW):
                wot = wots[dc]
                if dc + 2 < D // SCW:
                    load_wot(dc + 2)
                for ssub in range(S // 128):
                    s0 = ssub * 128
                    pd = ps.tile([128, SCW], F32, name=f"pd{dc}_{ssub}",
                                 tag="proj", bufs=2)
                    for kc8 in range(NQL):
                        nc.tensor.matmul(
                            pd[:],
                            attnT[kc8][:, s0:s0 + 128],
                            wot[:, kc8 * SCW:(kc8 + 1) * SCW],
                            start=(kc8 == 0), stop=(kc8 == NQL - 1))
                    os_ = sb.tile([128, SCW], BF16, name=f"os{dc}_{ssub}",
                                  tag="os", bufs=2)
                    nc.vector.tensor_copy(out=os_[:], in_=pd[:])
                    nc.sync.dma_start(
                        out=out[s0:s0 + 128, dc * SCW:(dc + 1) * SCW],
                        in_=os_[:])
    nc.finalize()
    return nc


_NC_CACHE = None


def _get_graph():
    global _NC_CACHE
    if _NC_CACHE is None:
        _NC_CACHE = _build()
    return _NC_CACHE


_PERM = np.concatenate([np.arange(0, HD, 2), np.arange(1, HD, 2)])


def _tile_w(w):
    """[D, M*HD] -> [128, m-major kc-major 128cols] contiguous tiling (bf16)."""
    d, mc = w.shape
    nm = mc // HD
    t = w.reshape(NKC, 128, nm, HD).transpose(1, 2, 0, 3)
    return np.ascontiguousarray(t.reshape(128, nm * NKC * HD).astype(NPBF))


def _tile_wo(w):
    """[NQL*HD, D] -> [128, dc-major kc-major 512cols] (bf16)."""
    t = w.reshape(NQL, 128, D // SCW, SCW).transpose(1, 2, 0, 3)
    return np.ascontiguousarray(
        t.reshape(128, (D // SCW) * NQL * SCW).astype(NPBF))


def _shard_inputs(x, freqs_cos, freqs_sin, wq, wk, wv, wo):
    """Build the 8 per-core input maps (pure numpy prep, nothing on-device)."""
    x = np.asarray(x, dtype=np.float32)
    wq = np.asarray(wq, dtype=np.float32)
    wk = np.asarray(wk, dtype=np.float32)
    wv = np.asarray(wv, dtype=np.float32)
    wo = np.asarray(wo, dtype=np.float32)
    cos = np.asarray(freqs_cos, dtype=np.float32)
    sin = np.asarray(freqs_sin, dtype=np.float32)

    # RoPE tables in on-chip [128, S] layout
    cos2 = np.empty((128, S), np.float32)
    sin2n = np.empty((128, S), np.float32)
    cos2[0:64] = cos.T
    cos2[64:128] = cos.T
    sin2n[0:64] = -sin.T
    sin2n[64:128] = sin.T
    cos2 = cos2.astype(NPBF)
    sin2n = sin2n.astype(NPBF)

    wq4 = wq.reshape(D, NH, HD)
    wk4 = wk.reshape(D, NKV, HD)
    wv4 = wv.reshape(D, NKV, HD)
    wo4 = wo.reshape(NH, HD, D)

    # x transposed + bf16 per batch
    xts = [np.ascontiguousarray(x[b].T.astype(NPBF)) for b in range(B)]

    in_maps = []
    for c in range(NCORES):
        b, g = divmod(c, TPG)
        qh = slice(g * NQL, (g + 1) * NQL)
        kvh = slice(g * NKVL, (g + 1) * NKVL)
        m = {
            "xt": xts[b],
            "wq": _tile_w(wq4[:, qh, :][:, :, _PERM].reshape(D, NQL * HD)),
            "wk": _tile_w(wk4[:, kvh, :][:, :, _PERM].reshape(D, NKVL * HD)),
            "wv": _tile_w(wv4[:, kvh, :].reshape(D, NKVL * HD)),
            "wo": _tile_wo(wo4[qh].reshape(NQL * HD, D)),
            "cosp": cos2,
            "sinp": sin2n,
        }
        in_maps.append(m)
    return in_maps


def kernel(x, start_pos, freqs_cos, freqs_sin, mask, wq, wk, wv, wo,
           cache_k, cache_v):
    x = np.asarray(x)
    in_maps = _shard_inputs(x, freqs_cos, freqs_sin, wq, wk, wv, wo)
    nc = _get_graph()
    res = run_bass_kernel_spmd(nc, in_maps, core_ids=list(range(NCORES)))
    out = np.zeros((B, S, D), dtype=np.float32)
    for b in range(B):
        acc = np.asarray(res.results[b * TPG]["out"]).astype(np.float32)
        for g in range(1, TPG):
            acc += np.asarray(res.results[b * TPG + g]["out"]).astype(np.float32)
        out[b] = acc
    return out
